# revision 54
# baseline (speedup 1.0000x reference)
"""Tensor-parallel attention kernel for Trainium2 (8 NeuronCores).

Problem: B=2, L=2048, DIM=1024, H=16 heads, HD=64 (QKV proj + RoPE + SDPA + out proj).

Sharding: tensor-parallel over heads — 2 heads per core, fp16 compute / fp32 psum.
Each core:
  - computes q/k feature-major (qT [128, 4096] = [2*64 hd, B*L]) via fp16
    matmuls of w_qkv column-slices against xT (host pre-transposed, fp16);
    k lands in two zero-padded buffers (kTzA rows 0:64 live / 64:128 zero,
    kTzB the reverse) so the score matmuls run as full 128x128-mode tiles
    with no PE mode switches,
  - V is computed directly token-major (stationary-swap: xt chunk as
    stationary, w_v as moving) — no PE transposes anywhere in the kernel,
  - RoPE: rotate_half on PE via a signed +-1 permutation matmul; psum
    evictions on ACT, muls/adds on DVE in fp16 (2x mode where eligible),
  - attention runs as ONE flat software-pipelined stream over all 8
    (batch, 512-query) tiles: for chunk i, scores(i) issue on PE, exp(i)
    on ACT (scale fused), and P^T@V(i-2) — the PV accumulation order is
    commutative, so deferring it two chunks hides the scores->exp->PV
    latency (~1.4us) under 8 matmuls of PE work; every FE_EVERY-th
    chunk's exp instead uses a fast exp2 bit-trick on DVE
    (round(a*s+b) -> int16, bitcast as fp16, ~+-3% sawtooth) to keep ACT
    off the critical path,
  - P^T @ V accumulates in fp32 psum over 16 key chunks, with a
    ones-column per head accumulating the softmax denominator (M=65),
  - per-qtile normalization + out-proj are deferred one qtile and spread
    across the next qtile's chunk stream in small steps (burst-issuing
    them congests DVE exactly when the next qtile's PV needs its psum
    evictions): fp16 reciprocal of d on DVE, a partition-scatter DMA to
    rows {0,64}, a broadcast matmul against a constant E matrix on PE,
    one in-place DVE mul, then out-proj matmuls against the w_out
    row-slice with psum evictions on DVE -> fp16 partial [4096, 1024].
Host sums the 8 partials in fp32 (the "all-reduce after out_proj").

Engine budget per qtile (sim): PE 15.3us (bottleneck), ACT ~13.1,
DVE ~13.3, Pool light (gpsimd cannot read PSUM). TimelineSim ~207us
vs 260us for the v1 kernel; no PE mode switches (the v1 kernel drained
the PE array twice per chunk switching 64-row-tiled scores <-> full-mode
PV, overhead the simulator does not model).
"""
import numpy as np

import concourse.bass as bass
import concourse.tile as tile
from concourse import bacc, mybir

B, L, DIM, H, HD = 2, 2048, 1024, 16, 64
NCORES = 8
HPC = H // NCORES            # heads per core = 2
T = B * L                    # 4096 tokens
NT = T // 512                # 8 token tiles of 512
KC = DIM // 128              # 8 contraction chunks for qkv
CH = T // 128                # 32 key chunks of 128 (global)
CHB = L // 128               # 16 key chunks per batch
QT = L // 512                # 4 query tiles per batch
VW = 2 * HD + 2              # v-nat chunk width: [V_A(64) | ones | V_B(64) | ones] = 130

F32 = mybir.dt.float32
F32R = mybir.dt.float32r
BF16 = mybir.dt.float16  # fp16: same PE rate as bf16, 8x mantissa
I16 = mybir.dt.int16
AF = mybir.ActivationFunctionType
ALU = mybir.AluOpType

# fast exp2 bit trick (fp16): exp(s/8) = 2^(s*0.125*log2 e);
# bits = round(1024*(t + 15 - 0.04303)) -> bitcast fp16. max rel err ~2.9%.
FE_A = 1024.0 * 0.125 * 1.4426950408889634
FE_B = 1024.0 * (15.0 - 0.04303)
# every FE_EVERY-th key chunk uses the DVE fast exp (both heads); the rest
# are exact on ACT. Output noise scales ~1.75% * sqrt(1/FE_EVERY).
FE_EVERY = 4

_CACHE = {}


def _grp2(ap2d, pair_stride, width):
    """Two-group free AP: columns {0..w-1, ps..ps+w-1} of a 2D [p, f] AP."""
    return bass.AP(tensor=ap2d.tensor, offset=ap2d.offset,
                   ap=[list(ap2d.ap[0]), [pair_stride, 2], [1, width]])


def _build_nc(reps=1, phases=(1, 2), io_lite=False, loop_reps=0):
    nc = bacc.Bacc("TRN2", target_bir_lowering=False, debug=False)

    if io_lite:
        # timing-only build: big tensors are device-internal (no per-call
        # host transfer); tiny dummy external IO keeps the runner shape.
        # Shape depends on loop_reps/reps so distinct timing builds never
        # collide in the NEFF cache (the key ignores the bass kernel body).
        dw = 128 + loop_reps + 16 * (reps - 1)
        dummy_in = nc.dram_tensor("dummy_in", [128, dw], F32, kind="ExternalInput")
        dummy_out = nc.dram_tensor("dummy_out", [128, dw], F32, kind="ExternalOutput")
        xT_d = nc.dram_tensor("xT", [DIM, T], BF16)
        wqkv_d = nc.dram_tensor("wqkv", [128, KC, 3, 128], BF16)
        wout_d = nc.dram_tensor("wout", [128, 2, 512], BF16)
        cos_d = nc.dram_tensor("cosr", [64, L], BF16)
        c2_d = nc.dram_tensor("c2r", [64, L], BF16)
        perm_d = nc.dram_tensor("perm", [128, 128], BF16)
        out_d = nc.dram_tensor("out", [T, DIM], BF16)
    else:
        xT_d = nc.dram_tensor("xT", [DIM, T], BF16, kind="ExternalInput")
        wqkv_d = nc.dram_tensor("wqkv", [128, KC, 3, 128], BF16, kind="ExternalInput")
        wout_d = nc.dram_tensor("wout", [128, 2, 512], BF16, kind="ExternalInput")
        cos_d = nc.dram_tensor("cosr", [64, L], BF16, kind="ExternalInput")
        c2_d = nc.dram_tensor("c2r", [64, L], BF16, kind="ExternalInput")
        perm_d = nc.dram_tensor("perm", [128, 128], BF16, kind="ExternalInput")
        out_d = nc.dram_tensor("out", [T, DIM], BF16, kind="ExternalOutput")

    def rep2(dram_ap):
        # DRAM [64, L] read twice -> stream of 128 rows (partition-doubling)
        return bass.AP(tensor=dram_ap.tensor, offset=dram_ap.offset,
                       ap=[[0, 2]] + list(dram_ap.ap))

    with tile.TileContext(nc) as tc:
        from contextlib import ExitStack
        outer = ExitStack()
        if io_lite:
            # the dummy pool must stay OPEN for the whole program: if it
            # exits early, the main pools alias its SBUF range and every
            # const write stalls ~7us on the dummy-out DMA — a gate the real
            # (graded) build doesn't have. Keep it off the sync queue too.
            dp = outer.enter_context(tc.tile_pool(name="dummy", bufs=1))
            dt_ = dp.tile([128, 128 + loop_reps + 16 * (reps - 1)], F32)
            nc.gpsimd.dma_start(out=dt_[:], in_=dummy_in[:])
            nc.gpsimd.dma_start(out=dummy_out[:], in_=dt_[:])
        loop_cm = tc.For_i(0, loop_reps, 1) if loop_reps else None
        if loop_cm is not None:
            loop_cm.__enter__()
        for _rep in range(reps):
          with ExitStack() as ctx:
            const = ctx.enter_context(tc.tile_pool(name="const", bufs=1))
            big = ctx.enter_context(tc.tile_pool(name="big", bufs=1))

            wqkv_sb = const.tile([128, KC, 3, 128], BF16)
            wout_sb = const.tile([128, 2, 512], BF16)
            cos_sb = const.tile([128, T], BF16)
            c2_sb = const.tile([128, T], BF16)
            perm_sb = const.tile([128, 128], BF16)

            # DMA-wire order at startup gates the first matmul. Engine-issued
            # hw-dma queues take ~6.6us to initialize and only the first few
            # sync-queue slots start immediately, so: wqkv takes sync slot 1,
            # xt0 slot 2 (issued in the nt=0 loop body below), and the
            # smaller consts follow from inside the first iteration. cos/c2
            # load once ([64, L]); partition doubling + the batch-1 copy are
            # SBUF-to-SBUF DMAs needed only ~10us in.
            wq_dma = nc.sync.dma_start(out=wqkv_sb[:], in_=wqkv_d[:])

            def load_consts():
                nc.sync.dma_start(out=perm_sb[:], in_=perm_d[:])
                for b in range(B):
                    sl = slice(b * L, (b + 1) * L)
                    nc.gpsimd.dma_start(out=cos_sb[:, sl], in_=rep2(cos_d[:]))
                    nc.gpsimd.dma_start(out=c2_sb[:, sl], in_=rep2(c2_d[:]))

            qT = big.tile([128, T], BF16)
            kTzA = big.tile([128, T], BF16)
            kTzB = big.tile([128, T], BF16)
            vnat = big.tile([128, CH, VW], BF16)
            OT = big.tile([128, T], BF16)
            # recip-denominator staging rows 0 (head A) / 64 (head B) + zeros
            # (engine writes need quadrant-aligned partition bases; the DMA
            # scatter below can hit partitions {0, 64} directly)
            ddr = big.tile([128, NT, 512], BF16)
            # E broadcast matrix: out row m <- ddr row 0 (m<64) / row 64 (m>=64)
            E_sb = const.tile([128, 128], BF16)

            # zero-pad the dead k halves once (full-mode score matmuls);
            # these follow the wqkv/perm descriptors in the DVE stream so the
            # weight loads hit the DMA wire first
            nc.vector.memset(kTzA[64:128, :], 0.0)
            nc.vector.memset(kTzB[0:64, :], 0.0)
            nc.vector.memset(ddr[:], 0.0)
            nc.gpsimd.memset(E_sb[:], 0.0)
            nc.gpsimd.memset(E_sb[0:1, 0:64], 1.0)
            nc.gpsimd.memset(E_sb[64:65, 64:128], 1.0)

            ones_sb = const.tile([128, CH], F32)
            nc.vector.memset(ones_sb[:], 1.0)
            nc.vector.tensor_copy(vnat[:, :, 64], ones_sb[:])
            nc.vector.tensor_copy(vnat[:, :, 129], ones_sb[:])

            # ---------------- Phase 1: QKV projection + RoPE + v token-major ----
            if 1 not in phases:
                continue
            with tc.tile_pool(name="qk_ps", bufs=2, space="PSUM") as qk_ps, \
                 tc.tile_pool(name="vt_ps", bufs=2, space="PSUM") as vt_ps, \
                 tc.tile_pool(name="rot_ps", bufs=2, space="PSUM") as rot_ps, \
                 tc.tile_pool(name="xt_pool", bufs=3) as xt_pool, \
                 tc.tile_pool(name="rope_pool", bufs=3) as rope_pool:

                for nt in range(NT):
                    csl = slice(nt * 512, (nt + 1) * 512)
                    psq = qk_ps.tile([128, 512], F32, tag="psq")
                    psk = qk_ps.tile([128, 512], F32, tag="psk")
                    xt = xt_pool.tile([128, KC, 512], BF16, tag="xt")
                    xt_dma = nc.sync.dma_start(
                        out=xt[:],
                        in_=xT_d[:, csl].rearrange("(c p) t -> p c t", p=128))
                    if nt == 0:
                        load_consts()
                    for kc in range(KC):
                        nc.tensor.matmul(psq[:], wqkv_sb[:, kc, 0, :], xt[:, kc, :],
                                         start=(kc == 0), stop=(kc == KC - 1))
                        nc.tensor.matmul(psk[:], wqkv_sb[:, kc, 1, :], xt[:, kc, :],
                                         start=(kc == 0), stop=(kc == KC - 1))

                    # raw psum evictions for RoPE issued before the v loop so
                    # ACT has them ready when PE reaches the perm matmuls
                    raws = {}
                    for which, ps in (("q", psq), ("k", psk)):
                        raw = rope_pool.tile([128, 512], BF16, tag="raw" + which)
                        nc.scalar.copy(raw[:], ps[:])
                        raws[which] = raw

                    # V token-major: stationary-swap (xt chunk stationary,
                    # w_v moving) -> [128 tokens, 128 vdims]; no transposes.
                    for j in range(4):
                        c = nt * 4 + j
                        jsl = slice(j * 128, (j + 1) * 128)
                        vt = vt_ps.tile([128, 128], F32, tag="vt")
                        for kc in range(KC):
                            nc.tensor.matmul(
                                vt[:], xt[:, kc, jsl], wqkv_sb[:, kc, 2, :],
                                start=(kc == 0), stop=(kc == KC - 1))
                        nc.scalar.copy(
                            _grp2(vnat[:, c, 0:VW], 65, 64),
                            _grp2(vt[:, 0:128], 64, 64))

                    # RoPE: dst = ps*cos + rot(ps)*c2, rot via +-1 perm matmul
                    # (c2 carries plain |sin|; signs live in perm).
                    for which, ps in (("q", psq), ("k", psk)):
                        raw = raws[which]
                        rps = rot_ps.tile([128, 512], F32, tag="rot")
                        nc.tensor.matmul(rps[:], perm_sb[:], raw[:])
                        m1 = rope_pool.tile([128, 512], BF16, tag="m1")
                        m2 = rope_pool.tile([128, 512], BF16, tag="m2")
                        nc.vector.tensor_mul(m1[:], raw[:], cos_sb[:, csl])
                        nc.vector.tensor_mul(m2[:], rps[:], c2_sb[:, csl])
                        if which == "q":
                            nc.vector.tensor_add(qT[:, csl], m1[:], m2[:])
                        else:
                            nc.vector.tensor_add(
                                kTzA[0:64, csl], m1[0:64, :], m2[0:64, :])
                            nc.vector.tensor_add(
                                kTzB[64:128, csl], m1[64:128, :], m2[64:128, :])

            nc.gpsimd.dma_start(out=wout_sb[:], in_=wout_d[:])

            # ---------------- Phase 2: attention + out projection ----------------
            if 2 not in phases:
                nc.sync.dma_start(out=out_d[0:128, :], in_=qT[:, 0:1024])
                continue
            with tc.tile_pool(name="s_ps", bufs=2, space="PSUM") as s_ps, \
                 tc.tile_pool(name="oa_ps", bufs=1, space="PSUM") as oa_ps, \
                 tc.tile_pool(name="ob_ps", bufs=1, space="PSUM") as ob_ps, \
                 tc.tile_pool(name="op_ps", bufs=2, space="PSUM") as op_ps, \
                 tc.tile_pool(name="pt_pool", bufs=5) as pt_pool, \
                 tc.tile_pool(name="d_pool", bufs=1) as d_pool, \
                 tc.tile_pool(name="rd_pool", bufs=2) as rd_pool, \
                 tc.tile_pool(name="o_pool", bufs=2) as o_pool, \
                 tc.tile_pool(name="st_pool", bufs=2) as st_pool:

                # d rows for all (b, qt, head): row 64 = denominators (fp16:
                # d in [~400, ~8000], well inside fp16 range)
                dall = d_pool.tile([65, 2, NT * 512 // 512, 512], BF16)

                NG = B * QT

                def finalize_steps(g, last=False):
                    # normalize OT slice for global qtile g and run its
                    # out-proj, as a list of small closures the caller spreads
                    # over the next qtile's chunk stream (issuing them in one
                    # burst congests DVE right when the next qtile's PV needs
                    # its psum evictions)
                    qsl = slice(g * 512, (g + 1) * 512)

                    def norm():
                        with nc.allow_low_precision(
                                reason="1/d fp16: d in [~400, ~8000]"):
                            nc.vector.reciprocal(
                                dall[64:65, :, g, :], dall[64:65, :, g, :])
                        dr = ddr[0:2, g, :]
                        scatter = bass.AP(
                            tensor=dr.tensor, offset=dr.offset,
                            ap=[[dr.ap[0][0] * 64, 2]]
                               + [list(d) for d in dr.ap[1:]])
                        with (tc.high_priority() if last and not loop_reps
                              else ExitStack()):
                            nc.sync.dma_start(
                                out=scatter, in_=dall[64:65, :, g, :])
                        rdfp = op_ps.tile([128, 512], F32, tag="po")
                        nc.tensor.matmul(rdfp[:], E_sb[:], ddr[:, g, :])
                        nc.vector.tensor_mul(OT[:, qsl], OT[:, qsl], rdfp[:])

                    steps = [norm]
                    stgs = {}

                    def op_piece(j2, a, dj):
                        def run():
                            if (a, dj) == (0, 0):
                                stg_new = st_pool.tile(
                                    [128, 2, 1024], BF16, tag="st")
                                stgs[j2] = stg_new
                            stg = stgs[j2]
                            tch0 = g * 4 + j2 * 2
                            tsl = slice((tch0 + a) * 128,
                                        (tch0 + a + 1) * 128)
                            po = op_ps.tile([128, 512], F32, tag="po")
                            nc.tensor.matmul(
                                po[:], OT[:, tsl], wout_sb[:, dj, :])
                            dst = stg[:, a, dj * 512:(dj + 1) * 512]
                            # gpsimd cannot read PSUM; DVE has the slack
                            # mid-stream, ACT helps on the final tail
                            if last and dj == 1:
                                nc.scalar.copy(dst, po[:])
                            else:
                                nc.vector.tensor_copy(dst, po[:])
                            if dj == 1:
                                nc.sync.dma_start(
                                    out=out_d[(tch0 + a) * 128:
                                              (tch0 + a + 1) * 128, :],
                                    in_=stg[:, a, :])
                        return run

                    for j2 in range(2):
                        for a in range(2):
                            for dj in range(2):
                                steps.append(op_piece(j2, a, dj))
                    return steps

                # one flat software-pipelined stream over all qtiles: PV runs
                # two chunks behind scores (accumulation is commutative), so
                # the scores->exp->PV latency hides under 8 matmuls of PE
                # work, and qtile boundaries don't drain the pipeline.
                pts = {}
                otiles = {}

                def scores_issue(i):
                    g, c = divmod(i, CHB)
                    b = g // QT
                    qsl = slice(g * 512, (g + 1) * 512)
                    cg = b * CHB + c
                    ksl = slice(cg * 128, (cg + 1) * 128)
                    s2 = s_ps.tile([128, 2, 512], F32, tag="s")
                    nc.tensor.matmul(s2[:, 0, :], kTzA[:, ksl], qT[:, qsl])
                    nc.tensor.matmul(s2[:, 1, :], kTzB[:, ksl], qT[:, qsl])
                    pt = pt_pool.tile([128, 2, 512], BF16, tag="pt")
                    pts[i] = pt
                    if "noexp" in phases:
                        nc.vector.tensor_copy(pt[:], s2[:])
                    elif c % FE_EVERY == FE_EVERY - 1 and i != NG * CHB - 1:
                        # fast bit-trick exp on DVE (keeps ACT free); the very
                        # last chunk stays on ACT so the tail's reciprocal
                        # isn't queued behind it on DVE
                        nc.vector.tensor_scalar(
                            pt[:].bitcast(I16), s2[:],
                            FE_A, FE_B, ALU.mult, ALU.add)
                    else:
                        nc.scalar.activation(
                            pt[:], s2[:], AF.Exp, scale=float(HD ** -0.5))

                def pv_issue(i):
                    if "nopv" in phases or i < 0:
                        return
                    g, c = divmod(i, CHB)
                    b = g // QT
                    qsl = slice(g * 512, (g + 1) * 512)
                    cg = b * CHB + c
                    first, last = (c == 0), (c == CHB - 1)
                    if first:
                        oA = oa_ps.tile([65, 512], F32, tag="oA")
                        oB = ob_ps.tile([65, 512], F32, tag="oB")
                        otiles[g] = (oA, oB)
                    oA, oB = otiles[g]
                    pt = pts.pop(i)
                    nc.tensor.matmul(oA[:], vnat[:, cg, 0:65], pt[:, 0, :],
                                     start=first, stop=last)
                    nc.tensor.matmul(oB[:], vnat[:, cg, 65:130], pt[:, 1, :],
                                     start=first, stop=last)
                    if not last:
                        return
                    # evict unnormalized O + d rows in parallel across
                    # engines so the single-buffered oA/oB psum banks free
                    # fast (the next qtile's PV stalls on them otherwise);
                    # d rows first so the last finalize's recip chain starts
                    # as early as possible
                    final_tile = (g == NG - 1)
                    nc.scalar.copy(dall[64:65, 0, g, :], oA[64:65, :])
                    nc.vector.tensor_copy(dall[64:65, 1, g, :], oB[64:65, :])
                    (nc.scalar.copy if final_tile else nc.vector.tensor_copy)(
                        OT[0:64, qsl], oA[0:64, :])
                    otb = o_pool.tile([64, 512], BF16, tag="otb")
                    (nc.scalar.copy if final_tile else nc.vector.tensor_copy)(
                        otb[:], oB[0:64, :])
                    nc.sync.dma_start(out=OT[64:128, qsl], in_=otb[:])
                    if "nofin" not in phases:
                        if g > 0:
                            pending.extend(finalize_steps(g - 1))
                        if final_tile:
                            pending.extend(finalize_steps(g, last=True))

                pending = []
                for i in range(NG * CHB):
                    scores_issue(i)
                    pv_issue(i - 3)
                    if pending:
                        pending.pop(0)()
                pv_issue(NG * CHB - 3)
                pv_issue(NG * CHB - 2)
                pv_issue(NG * CHB - 1)
                while pending:
                    pending.pop(0)()

        if loop_cm is not None:
            loop_cm.__exit__(None, None, None)
        outer.close()

    nc.compile()
    return nc


def _host_prep(x, cos, sin, w_qkv, w_out):
    x = np.asarray(x, dtype=np.float32)
    cos = np.asarray(cos, dtype=np.float32)
    sin = np.asarray(sin, dtype=np.float32)
    w_qkv = np.asarray(w_qkv, dtype=np.float32)
    w_out = np.asarray(w_out, dtype=np.float32)

    xT = np.ascontiguousarray(x.reshape(T, DIM).T).astype(np.float16)
    cosr = np.ascontiguousarray(cos.T).astype(np.float16)   # [64, L]
    c2 = np.ascontiguousarray(sin.T).astype(np.float16)      # [64, L] plain sin
    # rot(ps)[p] = -ps[p+32] (p%64<32), +ps[p-32] (p%64>=32), as perm.T @ ps:
    # matmul computes out[m,n] = sum_k perm[k,m]*ps[k,n] -> perm[j,p] = coeff.
    perm = np.zeros((128, 128), dtype=np.float32)  # cast to bf16 below
    for blk in range(2):
        b0 = blk * 64
        for p in range(32):
            perm[b0 + p + 32, b0 + p] = -1.0      # out p<32 <- -ps[p+32]
            perm[b0 + p, b0 + p + 32] = 1.0       # out p>=32 <- +ps[p-32]

    in_maps = []
    for c in range(NCORES):
        h0 = c * HPC
        fs = slice(h0 * HD, h0 * HD + HPC * HD)              # 128 feature cols
        wc = np.concatenate(
            [w_qkv[:, 0 * H * HD:][:, fs],
             w_qkv[:, 1 * H * HD:][:, fs],
             w_qkv[:, 2 * H * HD:][:, fs]], axis=1)          # [1024, 384] = q|k|v
        # [kc*128+p, m*128+f] -> [p, kc, m, f]
        wq = np.ascontiguousarray(
            wc.reshape(KC, 128, 3, 128).transpose(1, 0, 2, 3)).astype(np.float16)
        wo = np.ascontiguousarray(
            w_out[fs, :].reshape(128, 2, 512)).astype(np.float16)
        in_maps.append({
            "xT": xT, "wqkv": wq, "wout": wo, "cosr": cosr, "c2r": c2,
            "perm": perm.astype(np.float16),
        })
    return in_maps


def _get_runner():
    if "runner" in _CACHE:
        return _CACHE["runner"]

    import jax
    from jax.sharding import Mesh, PartitionSpec
    from jax.experimental.shard_map import shard_map
    from concourse import bass2jax

    nc = _build_nc()
    bass2jax.install_neuronx_cc_hook()

    in_names = ["xT", "wqkv", "wout", "cosr", "c2r", "perm"]
    out_names = ["out"]
    out_avals = [jax.core.ShapedArray((T, DIM), np.float16)]
    bind_names = in_names + out_names
    if nc.partition_id_tensor is not None:
        bind_names = bind_names + [nc.partition_id_tensor.name]

    def _body(*args):
        operands = list(args)
        if nc.partition_id_tensor is not None:
            operands.append(bass2jax.partition_id_tensor())
        outs = bass2jax._bass_exec_p.bind(
            *operands,
            out_avals=tuple(out_avals),
            in_names=tuple(bind_names),
            out_names=tuple(out_names),
            lowering_input_output_aliases=(),
            sim_require_finite=True,
            sim_require_nnan=True,
            nc=nc,
        )
        return tuple(outs)

    devices = jax.devices()[:NCORES]
    mesh = Mesh(np.asarray(devices), ("core",))
    in_specs = (PartitionSpec("core"),) * (len(in_names) + 1)
    out_specs = (PartitionSpec("core"),)
    sharded = jax.jit(
        shard_map(_body, mesh=mesh, in_specs=in_specs, out_specs=out_specs,
                  check_rep=False),
        donate_argnums=(len(in_names),),
        keep_unused=True,
    )
    _CACHE["runner"] = (sharded, in_names)
    return _CACHE["runner"]


def device_inputs(in_maps):
    """Concatenate per-core input maps along axis 0 in runner arg order."""
    _, in_names = _get_runner()
    return [
        np.concatenate([np.asarray(m[name]) for m in in_maps], axis=0)
        for name in in_names
    ]


def run_sharded(in_maps):
    """Run the SPMD kernel; returns list of per-core output arrays [T, DIM]."""
    sharded, _ = _get_runner()
    concat_in = device_inputs(in_maps)
    zeros = np.zeros((NCORES * T, DIM), np.float16)
    (out,) = sharded(*concat_in, zeros)
    out = np.asarray(out).reshape(NCORES, T, DIM)
    return [out[c] for c in range(NCORES)]


def kernel(x, cos, sin, w_qkv, w_out):
    in_maps = _host_prep(x, cos, sin, w_qkv, w_out)
    parts = run_sharded(in_maps)
    full = parts[0].astype(np.float32)
    for p in parts[1:]:
        full += p.astype(np.float32)
    return full.reshape(B, L, DIM)


if __name__ == "__main__":
    rng = np.random.default_rng(0)
    x = rng.standard_normal((B, L, DIM), dtype=np.float32)
    import reference
    inputs = reference.setup_inputs()
    out = kernel(**{k: np.asarray(v) for k, v in inputs.items()})
    ref = np.asarray(reference.reference(**inputs))
    err = np.abs(out - ref)
    rel = np.sqrt((err ** 2).mean()) / np.sqrt((ref ** 2).mean())
    print("rms rel:", rel, "max abs:", err.max())


# revision 67
# speedup vs baseline: 99.2657x; 99.2657x over previous
"""Tensor-parallel attention kernel for Trainium2 (8 NeuronCores).

Problem: B=2, L=2048, DIM=1024, H=16 heads, HD=64 (QKV proj + RoPE + SDPA + out proj).

Sharding: tensor-parallel over heads — 2 heads per core, fp16 compute / fp32 psum.
Each core:
  - computes q/k feature-major (qT [128, 4096] = [2*64 hd, B*L]) via fp16
    matmuls of w_qkv column-slices against xT (host pre-transposed, fp16);
    k lands in two zero-padded buffers (kTzA rows 0:64 live / 64:128 zero,
    kTzB the reverse) so the score matmuls run as full 128x128-mode tiles
    with no PE mode switches,
  - V is computed directly token-major (stationary-swap: xt chunk as
    stationary, w_v as moving) — no PE transposes anywhere in the kernel,
  - RoPE: rotate_half on PE via a signed +-1 permutation matmul; psum
    evictions on ACT, muls/adds on DVE in fp16 (2x mode where eligible),
  - attention runs as ONE flat software-pipelined stream over all 8
    (batch, 512-query) tiles: for chunk i, scores(i) issue on PE, exp(i)
    on ACT (scale fused), and P^T@V(i-2) — the PV accumulation order is
    commutative, so deferring it two chunks hides the scores->exp->PV
    latency (~1.4us) under 8 matmuls of PE work; every FE_EVERY-th
    chunk's exp instead uses a fast exp2 bit-trick on DVE
    (round(a*s+b) -> int16, bitcast as fp16, ~+-3% sawtooth) to keep ACT
    off the critical path,
  - P^T @ V accumulates in fp32 psum over 16 key chunks, with a
    ones-column per head accumulating the softmax denominator (M=65),
  - per-qtile normalization + out-proj are deferred one qtile and spread
    across the next qtile's chunk stream in small steps (burst-issuing
    them congests DVE exactly when the next qtile's PV needs its psum
    evictions): fp16 reciprocal of d on DVE, a partition-scatter DMA to
    rows {0,64}, a broadcast matmul against a constant E matrix on PE,
    one in-place DVE mul, then out-proj matmuls against the w_out
    row-slice with psum evictions on DVE -> fp16 partial [4096, 1024].
Host sums the 8 partials in fp32 (the "all-reduce after out_proj").

Engine budget per qtile (sim): PE 15.3us (bottleneck), ACT ~13.1,
DVE ~13.3, Pool light (gpsimd cannot read PSUM). TimelineSim ~207us
vs 260us for the v1 kernel; no PE mode switches (the v1 kernel drained
the PE array twice per chunk switching 64-row-tiled scores <-> full-mode
PV, overhead the simulator does not model).
"""
import numpy as np

import concourse.bass as bass
import concourse.tile as tile
from concourse import bacc, mybir

B, L, DIM, H, HD = 2, 2048, 1024, 16, 64
NCORES = 8
HPC = H // NCORES            # heads per core = 2
T = B * L                    # 4096 tokens
NT = T // 512                # 8 token tiles of 512
KC = DIM // 128              # 8 contraction chunks for qkv
CH = T // 128                # 32 key chunks of 128 (global)
CHB = L // 128               # 16 key chunks per batch
QT = L // 512                # 4 query tiles per batch
VW = 2 * HD + 2              # v-nat chunk width: [V_A(64) | ones | V_B(64) | ones] = 130

F32 = mybir.dt.float32
F32R = mybir.dt.float32r
BF16 = mybir.dt.float16  # fp16: same PE rate as bf16, 8x mantissa
I16 = mybir.dt.int16
AF = mybir.ActivationFunctionType
ALU = mybir.AluOpType

# fast exp2 bit trick (fp16): exp(s/8) = 2^(s*0.125*log2 e);
# bits = round(1024*(t + 15 - 0.04303)) -> bitcast fp16. max rel err ~2.9%.
FE_A = 1024.0 * 0.125 * 1.4426950408889634
FE_B = 1024.0 * (15.0 - 0.04303)
# every FE_EVERY-th key chunk uses the DVE fast exp (both heads); the rest
# are exact on ACT. Output noise scales ~1.75% * sqrt(1/FE_EVERY).
FE_EVERY = 4

_CACHE = {}


def _grp2(ap2d, pair_stride, width):
    """Two-group free AP: columns {0..w-1, ps..ps+w-1} of a 2D [p, f] AP."""
    return bass.AP(tensor=ap2d.tensor, offset=ap2d.offset,
                   ap=[list(ap2d.ap[0]), [pair_stride, 2], [1, width]])


def _build_nc(reps=1, phases=(1, 2), io_lite=False, loop_reps=0):
    nc = bacc.Bacc("TRN2", target_bir_lowering=False, debug=False)

    if io_lite:
        # timing-only build: big tensors are device-internal (no per-call
        # host transfer); tiny dummy external IO keeps the runner shape.
        # Shape depends on loop_reps/reps so distinct timing builds never
        # collide in the NEFF cache (the key ignores the bass kernel body).
        dw = 128 + loop_reps + 16 * (reps - 1)
        dummy_in = nc.dram_tensor("dummy_in", [128, dw], F32, kind="ExternalInput")
        dummy_out = nc.dram_tensor("dummy_out", [128, dw], F32, kind="ExternalOutput")
        xT_d = nc.dram_tensor("xT", [DIM, T], BF16)
        wqkv_d = nc.dram_tensor("wqkv", [128, KC, 3, 128], BF16)
        wout_d = nc.dram_tensor("wout", [128, 2, 512], BF16)
        cos_d = nc.dram_tensor("cosr", [64, L], BF16)
        c2_d = nc.dram_tensor("c2r", [64, L], BF16)
        perm_d = nc.dram_tensor("perm", [128, 128], BF16)
        out_d = nc.dram_tensor("out", [T, DIM], BF16)
    else:
        xT_d = nc.dram_tensor("xT", [DIM, T], BF16, kind="ExternalInput")
        wqkv_d = nc.dram_tensor("wqkv", [128, KC, 3, 128], BF16, kind="ExternalInput")
        wout_d = nc.dram_tensor("wout", [128, 2, 512], BF16, kind="ExternalInput")
        cos_d = nc.dram_tensor("cosr", [64, L], BF16, kind="ExternalInput")
        c2_d = nc.dram_tensor("c2r", [64, L], BF16, kind="ExternalInput")
        perm_d = nc.dram_tensor("perm", [128, 128], BF16, kind="ExternalInput")
        out_d = nc.dram_tensor("out", [T, DIM], BF16, kind="ExternalOutput")

    def rep2(dram_ap):
        # DRAM [64, L] read twice -> stream of 128 rows (partition-doubling)
        return bass.AP(tensor=dram_ap.tensor, offset=dram_ap.offset,
                       ap=[[0, 2]] + list(dram_ap.ap))

    with tile.TileContext(nc) as tc:
        from contextlib import ExitStack
        outer = ExitStack()
        if io_lite:
            # the dummy pool must stay OPEN for the whole program: if it
            # exits early, the main pools alias its SBUF range and every
            # const write stalls ~7us on the dummy-out DMA — a gate the real
            # (graded) build doesn't have. Keep it off the sync queue too.
            dp = outer.enter_context(tc.tile_pool(name="dummy", bufs=1))
            dt_ = dp.tile([128, 128 + loop_reps + 16 * (reps - 1)], F32)
            nc.gpsimd.dma_start(out=dt_[:], in_=dummy_in[:])
            nc.gpsimd.dma_start(out=dummy_out[:], in_=dt_[:])
        loop_cm = tc.For_i(0, loop_reps, 1) if loop_reps else None
        if loop_cm is not None:
            loop_cm.__enter__()
        for _rep in range(reps):
          with ExitStack() as ctx:
            const = ctx.enter_context(tc.tile_pool(name="const", bufs=1))
            big = ctx.enter_context(tc.tile_pool(name="big", bufs=1))

            wqkv_sb = const.tile([128, KC, 3, 128], BF16)
            wout_sb = const.tile([128, 2, 512], BF16)
            cos_sb = const.tile([128, T], BF16)
            c2_sb = const.tile([128, T], BF16)
            perm_sb = const.tile([128, 128], BF16)

            # DMA-wire order at startup gates the first matmul. Engine-issued
            # hw-dma queues take ~6.6us to initialize and only the first few
            # sync-queue slots start immediately, so: wqkv takes sync slot 1,
            # xt0 slot 2 (issued in the nt=0 loop body below), and the
            # smaller consts follow from inside the first iteration. cos/c2
            # load once ([64, L]); partition doubling + the batch-1 copy are
            # SBUF-to-SBUF DMAs needed only ~10us in.
            wq_dma = nc.sync.dma_start(out=wqkv_sb[:], in_=wqkv_d[:])

            def load_consts():
                nc.sync.dma_start(out=perm_sb[:], in_=perm_d[:])
                for b in range(B):
                    sl = slice(b * L, (b + 1) * L)
                    nc.gpsimd.dma_start(out=cos_sb[:, sl], in_=rep2(cos_d[:]))
                    nc.gpsimd.dma_start(out=c2_sb[:, sl], in_=rep2(c2_d[:]))

            qT = big.tile([128, T], BF16)
            kTzA = big.tile([128, T], BF16)
            kTzB = big.tile([128, T], BF16)
            vnat = big.tile([128, CH, VW], BF16)
            OT = big.tile([128, T], BF16)
            # recip-denominator staging rows 0 (head A) / 64 (head B) + zeros
            # (engine writes need quadrant-aligned partition bases; the DMA
            # scatter below can hit partitions {0, 64} directly)
            ddr = big.tile([128, NT, 512], BF16)
            # E broadcast matrix: out row m <- ddr row 0 (m<64) / row 64 (m>=64)
            E_sb = const.tile([128, 128], BF16)

            # zero-pad the dead k halves once (full-mode score matmuls);
            # these follow the wqkv/perm descriptors in the DVE stream so the
            # weight loads hit the DMA wire first
            nc.vector.memset(kTzA[64:128, :], 0.0)
            nc.vector.memset(kTzB[0:64, :], 0.0)
            nc.vector.memset(ddr[:], 0.0)
            nc.gpsimd.memset(E_sb[:], 0.0)
            nc.gpsimd.memset(E_sb[0:1, 0:64], 1.0)
            nc.gpsimd.memset(E_sb[64:65, 64:128], 1.0)

            ones_sb = const.tile([128, CH], F32)
            nc.vector.memset(ones_sb[:], 1.0)
            nc.vector.tensor_copy(vnat[:, :, 64], ones_sb[:])
            nc.vector.tensor_copy(vnat[:, :, 129], ones_sb[:])

            # ---------------- Phase 1: QKV projection + RoPE + v token-major ----
            if 1 not in phases:
                continue
            with tc.tile_pool(name="qk_ps", bufs=2, space="PSUM") as qk_ps, \
                 tc.tile_pool(name="vt_ps", bufs=2, space="PSUM") as vt_ps, \
                 tc.tile_pool(name="rot_ps", bufs=2, space="PSUM") as rot_ps, \
                 tc.tile_pool(name="xt_pool", bufs=3) as xt_pool, \
                 tc.tile_pool(name="rope_pool", bufs=3) as rope_pool:

                for nt in range(NT):
                    csl = slice(nt * 512, (nt + 1) * 512)
                    psq = qk_ps.tile([128, 512], F32, tag="psq")
                    psk = qk_ps.tile([128, 512], F32, tag="psk")
                    xt = xt_pool.tile([128, KC, 512], BF16, tag="xt")
                    xt_dma = nc.sync.dma_start(
                        out=xt[:],
                        in_=xT_d[:, csl].rearrange("(c p) t -> p c t", p=128))
                    if nt == 0:
                        load_consts()
                    for kc in range(KC):
                        nc.tensor.matmul(psq[:], wqkv_sb[:, kc, 0, :], xt[:, kc, :],
                                         start=(kc == 0), stop=(kc == KC - 1))
                        nc.tensor.matmul(psk[:], wqkv_sb[:, kc, 1, :], xt[:, kc, :],
                                         start=(kc == 0), stop=(kc == KC - 1))

                    # raw psum evictions for RoPE issued before the v loop so
                    # ACT has them ready when PE reaches the perm matmuls
                    raws = {}
                    for which, ps in (("q", psq), ("k", psk)):
                        raw = rope_pool.tile([128, 512], BF16, tag="raw" + which)
                        nc.scalar.copy(raw[:], ps[:])
                        raws[which] = raw

                    # V token-major: stationary-swap (xt chunk stationary,
                    # w_v moving) -> [128 tokens, 128 vdims]; no transposes.
                    for j in range(4):
                        c = nt * 4 + j
                        jsl = slice(j * 128, (j + 1) * 128)
                        vt = vt_ps.tile([128, 128], F32, tag="vt")
                        for kc in range(KC):
                            nc.tensor.matmul(
                                vt[:], xt[:, kc, jsl], wqkv_sb[:, kc, 2, :],
                                start=(kc == 0), stop=(kc == KC - 1))
                        nc.scalar.copy(
                            _grp2(vnat[:, c, 0:VW], 65, 64),
                            _grp2(vt[:, 0:128], 64, 64))

                    # RoPE: dst = ps*cos + rot(ps)*c2, rot via +-1 perm matmul
                    # (c2 carries plain |sin|; signs live in perm).
                    for which, ps in (("q", psq), ("k", psk)):
                        raw = raws[which]
                        rps = rot_ps.tile([128, 512], F32, tag="rot")
                        nc.tensor.matmul(rps[:], perm_sb[:], raw[:])
                        m1 = rope_pool.tile([128, 512], BF16, tag="m1")
                        m2 = rope_pool.tile([128, 512], BF16, tag="m2")
                        nc.vector.tensor_mul(m1[:], raw[:], cos_sb[:, csl])
                        nc.vector.tensor_mul(m2[:], rps[:], c2_sb[:, csl])
                        if which == "q":
                            nc.vector.tensor_add(qT[:, csl], m1[:], m2[:])
                        else:
                            nc.vector.tensor_add(
                                kTzA[0:64, csl], m1[0:64, :], m2[0:64, :])
                            nc.vector.tensor_add(
                                kTzB[64:128, csl], m1[64:128, :], m2[64:128, :])

            nc.gpsimd.dma_start(out=wout_sb[:], in_=wout_d[:])

            # ---------------- Phase 2: attention + out projection ----------------
            if 2 not in phases:
                nc.sync.dma_start(out=out_d[0:128, :], in_=qT[:, 0:1024])
                continue
            with tc.tile_pool(name="s_ps", bufs=2, space="PSUM") as s_ps, \
                 tc.tile_pool(name="oa_ps", bufs=1, space="PSUM") as oa_ps, \
                 tc.tile_pool(name="ob_ps", bufs=1, space="PSUM") as ob_ps, \
                 tc.tile_pool(name="op_ps", bufs=2, space="PSUM") as op_ps, \
                 tc.tile_pool(name="pt_pool", bufs=5) as pt_pool, \
                 tc.tile_pool(name="d_pool", bufs=1) as d_pool, \
                 tc.tile_pool(name="rd_pool", bufs=2) as rd_pool, \
                 tc.tile_pool(name="o_pool", bufs=2) as o_pool, \
                 tc.tile_pool(name="st_pool", bufs=2) as st_pool:

                # d rows for all (b, qt, head): row 64 = denominators (fp16:
                # d in [~400, ~8000], well inside fp16 range)
                dall = d_pool.tile([65, 2, NT * 512 // 512, 512], BF16)

                NG = B * QT

                def finalize_steps(g, last=False):
                    # normalize OT slice for global qtile g and run its
                    # out-proj, as a list of small closures the caller spreads
                    # over the next qtile's chunk stream (issuing them in one
                    # burst congests DVE right when the next qtile's PV needs
                    # its psum evictions)
                    qsl = slice(g * 512, (g + 1) * 512)

                    def norm():
                        with nc.allow_low_precision(
                                reason="1/d fp16: d in [~400, ~8000]"):
                            nc.vector.reciprocal(
                                dall[64:65, :, g, :], dall[64:65, :, g, :])
                        dr = ddr[0:2, g, :]
                        scatter = bass.AP(
                            tensor=dr.tensor, offset=dr.offset,
                            ap=[[dr.ap[0][0] * 64, 2]]
                               + [list(d) for d in dr.ap[1:]])
                        with (tc.high_priority() if last and not loop_reps
                              else ExitStack()):
                            nc.sync.dma_start(
                                out=scatter, in_=dall[64:65, :, g, :])
                        rdfp = op_ps.tile([128, 512], F32, tag="po")
                        nc.tensor.matmul(rdfp[:], E_sb[:], ddr[:, g, :])
                        nc.vector.tensor_mul(OT[:, qsl], OT[:, qsl], rdfp[:])

                    steps = [norm]
                    stgs = {}

                    def op_piece(j2, a, dj):
                        def run():
                            if (a, dj) == (0, 0):
                                stg_new = st_pool.tile(
                                    [128, 2, 1024], BF16, tag="st")
                                stgs[j2] = stg_new
                            stg = stgs[j2]
                            tch0 = g * 4 + j2 * 2
                            tsl = slice((tch0 + a) * 128,
                                        (tch0 + a + 1) * 128)
                            po = op_ps.tile([128, 512], F32, tag="po")
                            nc.tensor.matmul(
                                po[:], OT[:, tsl], wout_sb[:, dj, :])
                            dst = stg[:, a, dj * 512:(dj + 1) * 512]
                            # gpsimd cannot read PSUM; DVE has the slack
                            # mid-stream, ACT helps on the final tail
                            if last and dj == 1:
                                nc.scalar.copy(dst, po[:])
                            else:
                                nc.vector.tensor_copy(dst, po[:])
                            if dj == 1:
                                nc.sync.dma_start(
                                    out=out_d[(tch0 + a) * 128:
                                              (tch0 + a + 1) * 128, :],
                                    in_=stg[:, a, :])
                        return run

                    for j2 in range(2):
                        for a in range(2):
                            for dj in range(2):
                                steps.append(op_piece(j2, a, dj))
                    return steps

                # one flat software-pipelined stream over all qtiles: PV runs
                # two chunks behind scores (accumulation is commutative), so
                # the scores->exp->PV latency hides under 8 matmuls of PE
                # work, and qtile boundaries don't drain the pipeline.
                pts = {}
                otiles = {}

                def scores_issue(i):
                    g, c = divmod(i, CHB)
                    b = g // QT
                    qsl = slice(g * 512, (g + 1) * 512)
                    cg = b * CHB + c
                    ksl = slice(cg * 128, (cg + 1) * 128)
                    s2 = s_ps.tile([128, 2, 512], F32, tag="s")
                    nc.tensor.matmul(s2[:, 0, :], kTzA[:, ksl], qT[:, qsl])
                    nc.tensor.matmul(s2[:, 1, :], kTzB[:, ksl], qT[:, qsl])
                    pt = pt_pool.tile([128, 2, 512], BF16, tag="pt")
                    pts[i] = pt
                    if "noexp" in phases:
                        nc.vector.tensor_copy(pt[:], s2[:])
                    elif c % FE_EVERY == 1 and i != NG * CHB - 1:
                        # fast bit-trick exp on DVE (keeps ACT free); the very
                        # last chunk stays on ACT so the tail's reciprocal
                        # isn't queued behind it on DVE
                        nc.vector.tensor_scalar(
                            pt[:].bitcast(I16), s2[:],
                            FE_A, FE_B, ALU.mult, ALU.add)
                    else:
                        nc.scalar.activation(
                            pt[:], s2[:], AF.Exp, scale=float(HD ** -0.5))

                def pv_issue(i):
                    if "nopv" in phases or i < 0:
                        return
                    g, c = divmod(i, CHB)
                    b = g // QT
                    qsl = slice(g * 512, (g + 1) * 512)
                    cg = b * CHB + c
                    first, last = (c == 0), (c == CHB - 1)
                    if first:
                        oA = oa_ps.tile([65, 512], F32, tag="oA")
                        oB = ob_ps.tile([65, 512], F32, tag="oB")
                        otiles[g] = (oA, oB)
                    oA, oB = otiles[g]
                    pt = pts.pop(i)
                    nc.tensor.matmul(oA[:], vnat[:, cg, 0:65], pt[:, 0, :],
                                     start=first, stop=last)
                    nc.tensor.matmul(oB[:], vnat[:, cg, 65:130], pt[:, 1, :],
                                     start=first, stop=last)
                    if not last:
                        return
                    # evict unnormalized O + d rows in parallel across
                    # engines so the single-buffered oA/oB psum banks free
                    # fast (the next qtile's PV stalls on them otherwise);
                    # d rows first so the last finalize's recip chain starts
                    # as early as possible
                    final_tile = (g == NG - 1)
                    nc.scalar.copy(dall[64:65, 0, g, :], oA[64:65, :])
                    nc.vector.tensor_copy(dall[64:65, 1, g, :], oB[64:65, :])
                    (nc.scalar.copy if final_tile else nc.vector.tensor_copy)(
                        OT[0:64, qsl], oA[0:64, :])
                    otb = o_pool.tile([64, 512], BF16, tag="otb")
                    (nc.scalar.copy if final_tile else nc.vector.tensor_copy)(
                        otb[:], oB[0:64, :])
                    nc.sync.dma_start(out=OT[64:128, qsl], in_=otb[:])
                    if "nofin" not in phases:
                        if g > 0:
                            pending.extend(finalize_steps(g - 1))
                        if final_tile:
                            pending.extend(finalize_steps(g, last=True))

                pending = []
                for i in range(NG * CHB):
                    scores_issue(i)
                    pv_issue(i - 3)
                    if pending:
                        pending.pop(0)()
                pv_issue(NG * CHB - 3)
                pv_issue(NG * CHB - 2)
                pv_issue(NG * CHB - 1)
                while pending:
                    pending.pop(0)()

        if loop_cm is not None:
            loop_cm.__exit__(None, None, None)
        outer.close()

    nc.compile()
    return nc


def _host_prep(x, cos, sin, w_qkv, w_out):
    x = np.asarray(x, dtype=np.float32)
    cos = np.asarray(cos, dtype=np.float32)
    sin = np.asarray(sin, dtype=np.float32)
    w_qkv = np.asarray(w_qkv, dtype=np.float32)
    w_out = np.asarray(w_out, dtype=np.float32)

    xT = np.ascontiguousarray(x.reshape(T, DIM).T).astype(np.float16)
    cosr = np.ascontiguousarray(cos.T).astype(np.float16)   # [64, L]
    c2 = np.ascontiguousarray(sin.T).astype(np.float16)      # [64, L] plain sin
    # rot(ps)[p] = -ps[p+32] (p%64<32), +ps[p-32] (p%64>=32), as perm.T @ ps:
    # matmul computes out[m,n] = sum_k perm[k,m]*ps[k,n] -> perm[j,p] = coeff.
    perm = np.zeros((128, 128), dtype=np.float32)  # cast to bf16 below
    for blk in range(2):
        b0 = blk * 64
        for p in range(32):
            perm[b0 + p + 32, b0 + p] = -1.0      # out p<32 <- -ps[p+32]
            perm[b0 + p, b0 + p + 32] = 1.0       # out p>=32 <- +ps[p-32]

    in_maps = []
    for c in range(NCORES):
        h0 = c * HPC
        fs = slice(h0 * HD, h0 * HD + HPC * HD)              # 128 feature cols
        wc = np.concatenate(
            [w_qkv[:, 0 * H * HD:][:, fs],
             w_qkv[:, 1 * H * HD:][:, fs],
             w_qkv[:, 2 * H * HD:][:, fs]], axis=1)          # [1024, 384] = q|k|v
        # [kc*128+p, m*128+f] -> [p, kc, m, f]
        wq = np.ascontiguousarray(
            wc.reshape(KC, 128, 3, 128).transpose(1, 0, 2, 3)).astype(np.float16)
        wo = np.ascontiguousarray(
            w_out[fs, :].reshape(128, 2, 512)).astype(np.float16)
        in_maps.append({
            "xT": xT, "wqkv": wq, "wout": wo, "cosr": cosr, "c2r": c2,
            "perm": perm.astype(np.float16),
        })
    return in_maps


def _get_runner():
    if "runner" in _CACHE:
        return _CACHE["runner"]

    import jax
    from jax.sharding import Mesh, PartitionSpec
    from jax.experimental.shard_map import shard_map
    from concourse import bass2jax

    nc = _build_nc()
    bass2jax.install_neuronx_cc_hook()

    in_names = ["xT", "wqkv", "wout", "cosr", "c2r", "perm"]
    out_names = ["out"]
    out_avals = [jax.core.ShapedArray((T, DIM), np.float16)]
    bind_names = in_names + out_names
    if nc.partition_id_tensor is not None:
        bind_names = bind_names + [nc.partition_id_tensor.name]

    def _body(*args):
        operands = list(args)
        if nc.partition_id_tensor is not None:
            operands.append(bass2jax.partition_id_tensor())
        outs = bass2jax._bass_exec_p.bind(
            *operands,
            out_avals=tuple(out_avals),
            in_names=tuple(bind_names),
            out_names=tuple(out_names),
            lowering_input_output_aliases=(),
            sim_require_finite=True,
            sim_require_nnan=True,
            nc=nc,
        )
        return tuple(outs)

    devices = jax.devices()[:NCORES]
    mesh = Mesh(np.asarray(devices), ("core",))
    in_specs = (PartitionSpec("core"),) * (len(in_names) + 1)
    out_specs = (PartitionSpec("core"),)
    sharded = jax.jit(
        shard_map(_body, mesh=mesh, in_specs=in_specs, out_specs=out_specs,
                  check_rep=False),
        donate_argnums=(len(in_names),),
        keep_unused=True,
    )
    _CACHE["runner"] = (sharded, in_names)
    return _CACHE["runner"]


def device_inputs(in_maps):
    """Concatenate per-core input maps along axis 0 in runner arg order."""
    _, in_names = _get_runner()
    return [
        np.concatenate([np.asarray(m[name]) for m in in_maps], axis=0)
        for name in in_names
    ]


def run_sharded(in_maps):
    """Run the SPMD kernel; returns list of per-core output arrays [T, DIM]."""
    sharded, _ = _get_runner()
    concat_in = device_inputs(in_maps)
    zeros = np.zeros((NCORES * T, DIM), np.float16)
    (out,) = sharded(*concat_in, zeros)
    out = np.asarray(out).reshape(NCORES, T, DIM)
    return [out[c] for c in range(NCORES)]


def kernel(x, cos, sin, w_qkv, w_out):
    in_maps = _host_prep(x, cos, sin, w_qkv, w_out)
    parts = run_sharded(in_maps)
    full = parts[0].astype(np.float32)
    for p in parts[1:]:
        full += p.astype(np.float32)
    return full.reshape(B, L, DIM)


if __name__ == "__main__":
    rng = np.random.default_rng(0)
    x = rng.standard_normal((B, L, DIM), dtype=np.float32)
    import reference
    inputs = reference.setup_inputs()
    out = kernel(**{k: np.asarray(v) for k, v in inputs.items()})
    ref = np.asarray(reference.reference(**inputs))
    err = np.abs(out - ref)
    rel = np.sqrt((err ** 2).mean()) / np.sqrt((ref ** 2).mean())
    print("rms rel:", rel, "max abs:", err.max())
